# revision 1
# baseline (speedup 1.0000x reference)
"""Trainium2 Bass kernel for nn_ConvQuantizationWrapper.

The reference bit-slices an 8-bit quantized 3x3 conv into 32 (2-bit act x
1-bit weight) conv passes and recombines them with powers of two. That
decomposition exactly reconstructs

    out = conv2d(A, Wq) / (sa*sw) + bias
    A   = clip(round(x*sa - zp), 0, 255) + zp        (integers in [-128,127])
    Wq  = wrap_int8(round(w*sw))                     (integers in [-128,127])

in exact integer arithmetic (all partial sums < 2^24, so f32/bf16-input
matmuls are exact). The kernel therefore runs one quantized conv:

  - data-parallel over batch: 8 images per NeuronCore
  - per image pair: quantize on DVE (2 tensor_scalar ops; round via the
    +1.5*2^23 magic-number trick, replicating the reference's f32 rounding
    bit-exactly), bf16 result written into a zero-padded [58,58] layout
  - partition-swap copy so each image exists in both partition halves
  - 3x3 conv = 9 shifted [64,64] bf16 matmuls accumulated in PSUM,
    issued as 4 concurrent PE sub-tiles (2 row groups x 2 col groups =
    two images x two tap row-groups) for full 128x128 array utilization
  - epilogue on ACT: out = psum * (1/(sa*sw)) + bias  (per-channel bias)
"""

import numpy as np
import ml_dtypes

import concourse.bacc as bacc
import concourse.mybir as mybir
import concourse.tile as tile
from concourse import bass_utils

N_CORES = 8
IMGS = 8          # images per core (batch 64 / 8 cores)
C = 64
H = W = 56
HP = 58           # padded spatial
NPIX = H * W      # 3136
CHUNK_ROWS = 8
CHUNK = CHUNK_ROWS * W   # 448 output pixels per PSUM bank
NCHUNKS = H // CHUNK_ROWS
MAGIC = 12582912.0       # 1.5 * 2**23: float32 round-to-nearest-integer trick

_nc_cache = {}


def _build(sa: float, neg_zp: float, recip: float, reps: int = 1):
    """Build + compile the per-core Bass kernel (cached per scalar config)."""
    key = (sa, neg_zp, recip, reps)
    if key in _nc_cache:
        return _nc_cache[key]

    A = mybir.AluOpType
    nc = bacc.Bacc("TRN2", target_bir_lowering=False, debug=False)
    x_d = nc.dram_tensor("x", [IMGS, C, H, W], mybir.dt.float32,
                         kind="ExternalInput").ap()
    w_d = nc.dram_tensor("wt", [128, 9 * 64], mybir.dt.bfloat16,
                         kind="ExternalInput").ap()
    b_d = nc.dram_tensor("biasd", [128, 1], mybir.dt.float32,
                         kind="ExternalInput").ap()
    y_d = nc.dram_tensor("y", [IMGS, C, H, W], mybir.dt.float32,
                         kind="ExternalOutput").ap()

    taps = [(kh - 1, kw - 1) for kh in range(3) for kw in range(3)]

    with tile.TileContext(nc) as tc:
        with (
            tc.tile_pool(name="const", bufs=1) as cpool,
            tc.tile_pool(name="xbuf", bufs=1) as xpool,
            tc.tile_pool(name="work", bufs=2) as wpool,
            tc.tile_pool(name="psum", bufs=8, space="PSUM") as ppool,
        ):
            w_sb = cpool.tile([128, 9 * 64], mybir.dt.bfloat16, name="w_sb")
            nc.sync.dma_start(out=w_sb, in_=w_d)
            b_sb = cpool.tile([128, 1], mybir.dt.float32, name="b_sb")
            nc.sync.dma_start(out=b_sb, in_=b_d)

            # Persistent double-buffered quantized-image tiles.
            # X1 = [imgP on parts 0-63 ; imgQ on parts 64-127], X2 = swapped.
            Xbufs = []
            for j in range(2):
                X1 = xpool.tile([128, HP, HP], mybir.dt.bfloat16,
                                name=f"X1_{j}", tag=f"X1_{j}")
                X2 = xpool.tile([128, HP, HP], mybir.dt.bfloat16,
                                name=f"X2_{j}", tag=f"X2_{j}")
                # zero once: interior is rewritten every pair, border stays 0
                nc.vector.memset(X1.rearrange("p a b -> p (a b)"), 0.0)
                Xbufs.append((X1, X2))

            for rep in range(reps):
              for pair in range(IMGS // 2):
                X1, X2 = Xbufs[pair % 2]
                xf = wpool.tile([128, NPIX], mybir.dt.float32,
                                name="xf", tag="xf")
                nc.sync.dma_start(
                    out=xf,
                    in_=x_d[2 * pair:2 * pair + 2].rearrange(
                        "i c h w -> (i c) (h w)"))

                # t1 = (x * sa) + (-zp)   -- two chained f32 ALU ops, same
                # rounding sequence as the reference's x*sa - zp
                t1 = wpool.tile([128, NPIX], mybir.dt.float32,
                                name="t1", tag="t1")
                nc.vector.tensor_scalar(t1, xf, sa, neg_zp,
                                        op0=A.mult, op1=A.add)
                # A = round(t1) + zp  ->  bf16 into padded interior
                # (t1 + MAGIC) rounds to integer (RNE); subtract MAGIC+(-zp)
                nc.vector.tensor_scalar(
                    X1[:, 1:57, 1:57],
                    t1.rearrange("p (h w) -> p h w", h=H),
                    MAGIC, MAGIC + neg_zp,
                    op0=A.add, op1=A.subtract)
                # partition-swap duplicate (borders copied along -> zeros)
                nc.vector.tensor_copy(
                    X2[64:128].rearrange("p a b -> p (a b)"),
                    X1[0:64].rearrange("p a b -> p (a b)"))
                nc.vector.tensor_copy(
                    X2[0:64].rearrange("p a b -> p (a b)"),
                    X1[64:128].rearrange("p a b -> p (a b)"))

                ystage = wpool.tile([128, NPIX], mybir.dt.float32,
                                    name="ystage", tag="ystage")
                for ch in range(NCHUNKS):
                    ps = ppool.tile([128, CHUNK], mybir.dt.float32,
                                    name="ps", tag="ps")
                    # one row-group per accumulation group (HW requirement:
                    # mixed row-groups / interleaved groups in one bank
                    # hang); alternate per chunk+pair for 4-quadrant balance
                    rg = 64 * ((ch + pair) % 2)
                    bufP = X1 if rg == 0 else X2
                    bufQ = X2 if rg == 0 else X1
                    for half, buf in ((0, bufP), (64, bufQ)):
                        for t in range(9):
                            dh, dw = taps[t]
                            rs = CHUNK_ROWS * ch + 1 + dh
                            cs = 1 + dw
                            lhsT = w_sb[rg:rg + 64, t * 64:(t + 1) * 64]
                            mov = buf[rg:rg + 64, rs:rs + 8, cs:cs + 56]
                            nc.tensor.matmul(ps[half:half + 64], lhsT, mov,
                                             start=(t == 0), stop=(t == 8))
                    # epilogue: y = psum * recip + bias (per-partition)
                    nc.scalar.activation(
                        out=ystage[:, ch * CHUNK:(ch + 1) * CHUNK],
                        in_=ps,
                        func=mybir.ActivationFunctionType.Identity,
                        bias=b_sb, scale=recip)

                nc.sync.dma_start(
                    out=y_d[2 * pair:2 * pair + 2].rearrange(
                        "i c h w -> (i c) (h w)"),
                    in_=ystage)

    nc.compile()
    _nc_cache[key] = nc
    return nc


def _prep(x, weight, bias, scale_a, scale_w, zero_point):
    x = np.ascontiguousarray(np.asarray(x, dtype=np.float32))
    weight = np.asarray(weight, dtype=np.float32)
    bias = np.asarray(bias, dtype=np.float32)
    sa = float(np.asarray(scale_a).reshape(-1)[0])
    sw = float(np.asarray(scale_w).reshape(-1)[0])
    zp = float(np.asarray(zero_point).reshape(-1)[0])

    # activation-clip guard: reference clips round(x*sa - zp) to [0, 255].
    # For in-range data the clip is a no-op; if any value could clip,
    # pre-clamp x on the host (preserves the reference's semantics).
    amax = float(np.abs(x).max())
    if not (amax * abs(sa) < abs(zp if zp != 0 else 0) + 126.99 and
            -0.49 < -zp and sa * amax - zp < 255.49):
        f32 = np.float32
        lo = (f32(-0.49) + f32(zp)) / f32(sa)
        hi = (f32(255.49) + f32(zp)) / f32(sa)
        x = np.clip(x, lo, hi).astype(np.float32)

    # weight quantization, matching jnp.round(weight * sw) in f32 + the
    # implicit 8-bit two's-complement wrap of the bit decomposition
    qw = np.round(weight * np.float32(sw))
    qwi = qw.astype(np.int64)
    qw_eff = ((qwi + 128) % 256) - 128
    delta = qwi - qw_eff          # nonzero only if |qw| > 127 (never for
    # randn*20 weights); handled via a host-side correction plane below.

    wt = qw_eff.astype(np.float32).transpose(1, 2, 3, 0).reshape(C, 9 * C)
    wt_dup = np.ascontiguousarray(
        np.concatenate([wt, wt], axis=0)).astype(ml_dtypes.bfloat16)
    bias_dup = np.ascontiguousarray(
        np.concatenate([bias, bias])[:, None].astype(np.float32))

    sprod = np.float32(sw) * np.float32(sa)
    recip = float(np.float32(1.0) / sprod)

    corr = None
    if np.any(delta != 0):
        # reference's zero-point term uses the unwrapped qw:
        # out_ref - out_dev = zp * conv2d(ones, delta) * recip
        dsum = delta.sum(axis=1).astype(np.float64)  # [o, 3, 3]
        plane = np.zeros((C, H, W), np.float64)
        for kh in range(3):
            for kw in range(3):
                h0, h1 = max(0, 1 - kh), min(H, H + 1 - kh)
                w0, w1 = max(0, 1 - kw), min(W, W + 1 - kw)
                plane[:, h0:h1, w0:w1] += dsum[:, kh, kw][:, None, None]
        corr = (zp * plane * float(recip)).astype(np.float32)

    return x, wt_dup, bias_dup, sa, zp, recip, corr


def _run(x, weight, bias, scale_a, scale_w, zero_point, trace=False):
    x, wt_dup, bias_dup, sa, zp, recip, corr = _prep(
        x, weight, bias, scale_a, scale_w, zero_point)
    nc = _build(sa, -zp, recip)
    n = x.shape[0]
    assert n == N_CORES * IMGS, f"expected batch {N_CORES * IMGS}, got {n}"
    in_maps = [
        {"x": np.ascontiguousarray(x[k * IMGS:(k + 1) * IMGS]),
         "wt": wt_dup, "biasd": bias_dup}
        for k in range(N_CORES)
    ]
    try:
        res = bass_utils.run_bass_kernel_spmd(
            nc, in_maps, core_ids=list(range(N_CORES)), trace=trace)
    except ModuleNotFoundError:
        # axon NTFF profile hook unavailable in this environment
        res = bass_utils.run_bass_kernel_spmd(
            nc, in_maps, core_ids=list(range(N_CORES)), trace=False)
    y = np.concatenate([res.results[k]["y"] for k in range(N_CORES)], axis=0)
    if corr is not None:
        y = y + corr[None]
    return np.ascontiguousarray(y.astype(np.float32)), res


def kernel(x, weight, bias, scale_a, scale_w, zero_point):
    y, _ = _run(x, weight, bias, scale_a, scale_w, zero_point, trace=False)
    return y



# revision 4
# speedup vs baseline: 1.5832x; 1.5832x over previous
"""Trainium2 Bass kernel for nn_ConvQuantizationWrapper.

The reference bit-slices an 8-bit quantized 3x3 conv into 32 (2-bit act x
1-bit weight) conv passes and recombines them with powers of two. That
decomposition exactly reconstructs

    out = conv2d(A, Wq) / (sa*sw) + bias
    A   = clip(round(x*sa - zp), 0, 255) + zp        (integers in [-128,127])
    Wq  = wrap_int8(round(w*sw))                     (integers in [-128,127])

in exact integer arithmetic (all partial sums < 2^24, so bf16-input
matmuls with fp32 PSUM accumulation are exact). The kernel runs one
quantized conv, data-parallel over batch (8 images per NeuronCore).

Per image pair (2 images = 128 partitions of staging):
  - ACT: u = x*sa + (MAGIC - zp)  (fused affine; MAGIC forces RNE-to-int)
  - DVE: A-slot writes  A = u - (MAGIC - zp) -> bf16 padded [58,58] frames
         plus 3 shifted copies (all even column shifts -> 4x DVE mode):
           T1 = [A ; B=A shifted (1,0)]   T2 = [C=A shifted (2,0) ; C2=(2,2)]
  - PE: 3x3 conv as 5 matmul groups per 8-row output chunk (K=128
        contraction = 64ch x 2 taps, vs 9 K=64 matmuls naively):
           g0..g2: taps {(0,q),(1,q)} from T1 at col offset q
           g3:     taps {(2,0),(2,2)} from T2 at col offset 0
           g4:     tap  {(2,1)} from T2 at col offset 1 (upper half zero wts)
        Per chunk/bank: img0's 5-matmul accumulation group completes, then
        img1's (never two groups open in one bank; uniform K=128 rows).
  - ACT epilogue: y = psum * (1/(sa*sw)) + bias
"""

import numpy as np
import ml_dtypes

import concourse.bacc as bacc
import concourse.mybir as mybir
import concourse.tile as tile
from concourse import bass_utils

N_CORES = 8
IMGS = 8          # images per core (batch 64 / 8 cores)
C = 64
H = W = 56
HP = 58           # padded spatial
NPIX = H * W      # 3136
CHUNK_ROWS = 8
CHUNK = CHUNK_ROWS * W   # 448 output pixels per PSUM bank
NCHUNKS = H // CHUNK_ROWS
MAGIC = 12582912.0       # 1.5 * 2**23: float32 round-to-nearest-integer trick
NGROUPS = 5

_nc_cache = {}


def _build(sa: float, neg_zp: float, recip: float):
    """Build + compile the per-core Bass kernel (cached per scalar config)."""
    key = (sa, neg_zp, recip)
    if key in _nc_cache:
        return _nc_cache[key]

    A = mybir.AluOpType
    F = mybir.ActivationFunctionType
    nc = bacc.Bacc("TRN2", target_bir_lowering=False, debug=False)
    x_d = nc.dram_tensor("x", [IMGS, C, H, W], mybir.dt.float32,
                         kind="ExternalInput").ap()
    w_d = nc.dram_tensor("wt", [128, NGROUPS * 64], mybir.dt.bfloat16,
                         kind="ExternalInput").ap()
    b_d = nc.dram_tensor("biasd", [128, 1], mybir.dt.float32,
                         kind="ExternalInput").ap()
    y_d = nc.dram_tensor("y", [IMGS, C, H, W], mybir.dt.float32,
                         kind="ExternalOutput").ap()

    # u = x*sa + (MAGIC - zp); A-slot = u - (MAGIC - zp)
    u_bias = MAGIC + neg_zp          # MAGIC - zp
    a_bias = -(MAGIC + neg_zp)

    with tile.TileContext(nc) as tc:
        with (
            tc.tile_pool(name="const", bufs=1) as cpool,
            tc.tile_pool(name="xbuf", bufs=1) as xpool,
            tc.tile_pool(name="work", bufs=2) as wpool,
            tc.tile_pool(name="psum", bufs=8, space="PSUM") as ppool,
        ):
            w_sb = cpool.tile([128, NGROUPS * 64], mybir.dt.bfloat16,
                              name="w_sb")
            nc.sync.dma_start(out=w_sb, in_=w_d)
            b_sb = cpool.tile([128, 1], mybir.dt.float32, name="b_sb")
            nc.sync.dma_start(out=b_sb, in_=b_d)
            ub_sb = cpool.tile([128, 1], mybir.dt.float32, name="ub_sb")
            nc.vector.memset(ub_sb, u_bias)

            # Persistent double-buffered layout tiles, [tap-slot halves,
            # img, 58, 58].  Zeroed once; interiors rewritten per pair,
            # borders stay 0.
            Tbufs = []
            for j in range(2):
                T1 = xpool.tile([128, 2, HP, HP], mybir.dt.bfloat16,
                                name=f"T1_{j}", tag=f"T1_{j}")
                T2 = xpool.tile([128, 2, HP, HP], mybir.dt.bfloat16,
                                name=f"T2_{j}", tag=f"T2_{j}")
                nc.vector.memset(T1.rearrange("p a b c -> p (a b c)"), 0.0)
                nc.vector.memset(T2.rearrange("p a b c -> p (a b c)"), 0.0)
                Tbufs.append((T1, T2))

            for pair in range(IMGS // 2):
                T1, T2 = Tbufs[pair % 2]
                xf = wpool.tile([128, NPIX], mybir.dt.float32,
                                name="xf", tag="xf")
                nc.sync.dma_start(
                    out=xf,
                    in_=x_d[2 * pair:2 * pair + 2].rearrange(
                        "i c h w -> (i c) (h w)"))

                # u = x*sa + (MAGIC - zp)  on ACT (frees DVE for copies)
                u = wpool.tile([128, NPIX], mybir.dt.float32,
                               name="u", tag="u")
                nc.scalar.activation(out=u, in_=xf, func=F.Identity,
                                     bias=ub_sb, scale=sa)

                # A-slot interiors (bf16), one DVE op per image
                for i in range(2):
                    nc.vector.tensor_scalar(
                        T1[0:64, i:i + 1, 1:57, 1:57],
                        u[64 * i:64 * i + 64].rearrange(
                            "p (h w) -> p h w", h=H),
                        a_bias, None, op0=A.add)
                # shifted copies (even col shifts -> 4x DVE mode)
                nc.vector.tensor_copy(          # B = A shift (1,0)
                    T1[64:128, :, 0:57, :],
                    T1[0:64, :, 1:58, :])
                nc.vector.tensor_copy(          # C = A shift (2,0)
                    T2[0:64, :, 0:56, :],
                    T1[0:64, :, 2:58, :])
                nc.vector.tensor_copy(          # C2 = A shift (2,2)
                    T2[64:128, :, 0:56, 0:56],
                    T1[0:64, :, 2:58, 2:58])

                ystage = wpool.tile([128, NPIX], mybir.dt.float32,
                                    name="ystage", tag="ystage")
                for ch in range(NCHUNKS):
                    R = CHUNK_ROWS * ch
                    ps = ppool.tile([128, CHUNK], mybir.dt.float32,
                                    name="ps", tag="ps")
                    # per chunk: img0's full 5-group accumulation, then
                    # img1's (one open group per bank at a time)
                    for half, i in ((0, 0), (64, 1)):
                        for g in range(NGROUPS):
                            src = T1 if g < 3 else T2
                            c0 = (g if g < 3 else g - 3)
                            mov = src[0:128, i:i + 1, R:R + CHUNK_ROWS,
                                      c0:c0 + 56]
                            lhsT = w_sb[0:128, g * 64:(g + 1) * 64]
                            nc.tensor.matmul(ps[half:half + 64], lhsT, mov,
                                             start=(g == 0),
                                             stop=(g == NGROUPS - 1))
                    # epilogue: y = psum * recip + bias (per-partition)
                    nc.scalar.activation(
                        out=ystage[:, ch * CHUNK:(ch + 1) * CHUNK],
                        in_=ps,
                        func=F.Identity,
                        bias=b_sb, scale=recip)

                nc.sync.dma_start(
                    out=y_d[2 * pair:2 * pair + 2].rearrange(
                        "i c h w -> (i c) (h w)"),
                    in_=ystage)

    nc.compile()
    _nc_cache[key] = nc
    return nc


def _prep(x, weight, bias, scale_a, scale_w, zero_point):
    x = np.ascontiguousarray(np.asarray(x, dtype=np.float32))
    weight = np.asarray(weight, dtype=np.float32)
    bias = np.asarray(bias, dtype=np.float32)
    sa = float(np.asarray(scale_a).reshape(-1)[0])
    sw = float(np.asarray(scale_w).reshape(-1)[0])
    zp = float(np.asarray(zero_point).reshape(-1)[0])

    # activation-clip guard: reference clips round(x*sa - zp) to [0, 255].
    # For in-range data the clip is a no-op; if any value could clip,
    # pre-clamp x on the host (preserves the reference's semantics).
    amax = float(np.abs(x).max())
    if not (amax * abs(sa) < abs(zp if zp != 0 else 0) + 126.99 and
            -0.49 < -zp and sa * amax - zp < 255.49):
        f32 = np.float32
        lo = (f32(-0.49) + f32(zp)) / f32(sa)
        hi = (f32(255.49) + f32(zp)) / f32(sa)
        x = np.clip(x, lo, hi).astype(np.float32)

    # weight quantization, matching jnp.round(weight * sw) in f32 + the
    # implicit 8-bit two's-complement wrap of the bit decomposition
    qw = np.round(weight * np.float32(sw))
    qwi = qw.astype(np.int64)
    qw_eff = ((qwi + 128) % 256) - 128
    delta = qwi - qw_eff          # nonzero only if |qw| > 127 (never for
    # randn*20 weights); handled via a host-side correction plane below.

    wt = qw_eff.astype(np.float32)      # [o, i, 3, 3]

    def tap(kh, kw):
        return np.ascontiguousarray(wt[:, :, kh, kw].T)   # [in, out]

    wg = np.zeros((128, NGROUPS * 64), np.float32)
    for g in range(3):                   # {(0,g),(1,g)} pairs
        wg[0:64, g * 64:(g + 1) * 64] = tap(0, g)
        wg[64:128, g * 64:(g + 1) * 64] = tap(1, g)
    wg[0:64, 192:256] = tap(2, 0)        # g3: {(2,0),(2,2)}
    wg[64:128, 192:256] = tap(2, 2)
    wg[0:64, 256:320] = tap(2, 1)        # g4: single {(2,1)}, upper half 0
    wg_bf = np.ascontiguousarray(wg.astype(ml_dtypes.bfloat16))

    bias_dup = np.ascontiguousarray(
        np.concatenate([bias, bias])[:, None].astype(np.float32))

    sprod = np.float32(sw) * np.float32(sa)
    recip = float(np.float32(1.0) / sprod)

    corr = None
    if np.any(delta != 0):
        # reference's zero-point term uses the unwrapped qw:
        # out_ref - out_dev = zp * conv2d(ones, delta) * recip
        dsum = delta.sum(axis=1).astype(np.float64)  # [o, 3, 3]
        plane = np.zeros((C, H, W), np.float64)
        for kh in range(3):
            for kw in range(3):
                h0, h1 = max(0, 1 - kh), min(H, H + 1 - kh)
                w0, w1 = max(0, 1 - kw), min(W, W + 1 - kw)
                plane[:, h0:h1, w0:w1] += dsum[:, kh, kw][:, None, None]
        corr = (zp * plane * float(recip)).astype(np.float32)

    return x, wg_bf, bias_dup, sa, zp, recip, corr


def _run(x, weight, bias, scale_a, scale_w, zero_point, trace=False):
    x, wg_bf, bias_dup, sa, zp, recip, corr = _prep(
        x, weight, bias, scale_a, scale_w, zero_point)
    nc = _build(sa, -zp, recip)
    n = x.shape[0]
    assert n == N_CORES * IMGS, f"expected batch {N_CORES * IMGS}, got {n}"
    in_maps = [
        {"x": np.ascontiguousarray(x[k * IMGS:(k + 1) * IMGS]),
         "wt": wg_bf, "biasd": bias_dup}
        for k in range(N_CORES)
    ]
    try:
        res = bass_utils.run_bass_kernel_spmd(
            nc, in_maps, core_ids=list(range(N_CORES)), trace=trace)
    except ModuleNotFoundError:
        # axon NTFF profile hook unavailable in this environment
        res = bass_utils.run_bass_kernel_spmd(
            nc, in_maps, core_ids=list(range(N_CORES)), trace=False)
    y = np.concatenate([res.results[k]["y"] for k in range(N_CORES)], axis=0)
    if corr is not None:
        y = y + corr[None]
    return np.ascontiguousarray(y.astype(np.float32)), res


def kernel(x, weight, bias, scale_a, scale_w, zero_point):
    y, _ = _run(x, weight, bias, scale_a, scale_w, zero_point, trace=False)
    return y


# revision 5
# speedup vs baseline: 1.9723x; 1.2458x over previous
"""Trainium2 Bass kernel for nn_ConvQuantizationWrapper.

The reference bit-slices an 8-bit quantized 3x3 conv into 32 (2-bit act x
1-bit weight) conv passes and recombines them with powers of two. That
decomposition exactly reconstructs

    out = conv2d(A, Wq) / (sa*sw) + bias
    A   = clip(round(x*sa - zp), 0, 255) + zp        (integers in [-128,127])
    Wq  = wrap_int8(round(w*sw))                     (integers in [-128,127])

in exact integer arithmetic (all partial sums < 2^24, so bf16-input
matmuls with fp32 PSUM accumulation are exact). The kernel runs one
quantized conv, data-parallel over batch (8 images per NeuronCore).

Per image pair (2 images = 128 partitions of staging):
  - ACT: u = x*sa + (MAGIC - zp)  (fused affine; MAGIC forces RNE-to-int)
  - DVE: A-slot writes  A = u - (MAGIC - zp) -> bf16 padded [58,58] frames
         in T1[0:64], plus one row-shifted copy B = A shift (1,0) into
         T1[64:128] (K=128 tap pairing needs taps (kh),(kh+1) at the same
         within-partition offset).
  - PE: 3x3 conv as 6 matmul groups per 8-row output chunk, all K=128
        (64ch x 2 kernel rows) x M=64 x N=448:
           g0..g2: taps {(0,q),(1,q)}  from T1 at (R,   q)
           g3..g5: tap  {(2,q)} via B-half at (R+1, q), A-half weights = 0
        Per chunk/bank: img0's full 6-matmul accumulation group completes,
        then img1's (never two groups open in one bank; uniform rows).
  - ACT epilogue: y = psum * (1/(sa*sw)) + bias
"""

import numpy as np
import ml_dtypes

import concourse.bacc as bacc
import concourse.mybir as mybir
import concourse.tile as tile
from concourse import bass_utils

N_CORES = 8
IMGS = 8          # images per core (batch 64 / 8 cores)
C = 64
H = W = 56
HP = 58           # padded spatial
NPIX = H * W      # 3136
CHUNK_ROWS = 8
CHUNK = CHUNK_ROWS * W   # 448 output pixels per PSUM bank
NCHUNKS = H // CHUNK_ROWS
MAGIC = 12582912.0       # 1.5 * 2**23: float32 round-to-nearest-integer trick
NGROUPS = 6

_nc_cache = {}


def _build(sa: float, neg_zp: float, recip: float):
    """Build + compile the per-core Bass kernel (cached per scalar config)."""
    key = (sa, neg_zp, recip)
    if key in _nc_cache:
        return _nc_cache[key]

    A = mybir.AluOpType
    F = mybir.ActivationFunctionType
    nc = bacc.Bacc("TRN2", target_bir_lowering=False, debug=False)
    x_d = nc.dram_tensor("x", [IMGS, C, H, W], mybir.dt.float32,
                         kind="ExternalInput").ap()
    w_d = nc.dram_tensor("wt", [128, NGROUPS * 64], mybir.dt.bfloat16,
                         kind="ExternalInput").ap()
    b_d = nc.dram_tensor("biasd", [128, 1], mybir.dt.float32,
                         kind="ExternalInput").ap()
    y_d = nc.dram_tensor("y", [IMGS, C, H, W], mybir.dt.float32,
                         kind="ExternalOutput").ap()

    # u = x*sa + (MAGIC - zp); A-slot = u - (MAGIC - zp)
    u_bias = MAGIC + neg_zp          # MAGIC - zp
    a_bias = -(MAGIC + neg_zp)

    with tile.TileContext(nc) as tc:
        with (
            tc.tile_pool(name="const", bufs=1) as cpool,
            tc.tile_pool(name="xbuf", bufs=1) as xpool,
            tc.tile_pool(name="xin", bufs=3) as xinpool,
            tc.tile_pool(name="work", bufs=2) as wpool,
            tc.tile_pool(name="psum", bufs=8, space="PSUM") as ppool,
        ):
            w_sb = cpool.tile([128, NGROUPS * 64], mybir.dt.bfloat16,
                              name="w_sb")
            nc.sync.dma_start(out=w_sb, in_=w_d)
            b_sb = cpool.tile([128, 1], mybir.dt.float32, name="b_sb")
            nc.sync.dma_start(out=b_sb, in_=b_d)
            ub_sb = cpool.tile([128, 1], mybir.dt.float32, name="ub_sb")
            nc.vector.memset(ub_sb, u_bias)

            # Persistent double-buffered layout tiles [slot-half, img, 58,58].
            # Only the A-half borders need zeroing (reads of the pad frame);
            # the B-half is fully covered by the shifted copy each pair.
            Tbufs = []
            for j in range(2):
                T1 = xpool.tile([128, 2, HP, HP], mybir.dt.bfloat16,
                                name=f"T1_{j}", tag=f"T1_{j}")
                nc.vector.memset(T1[0:64, :, 0:1, :], 0.0)    # top pad row
                nc.vector.memset(T1[0:64, :, 57:58, :], 0.0)  # bottom pad row
                nc.vector.memset(T1[0:64, :, :, 0:1], 0.0)    # left pad col
                nc.vector.memset(T1[0:64, :, :, 57:58], 0.0)  # right pad col
                Tbufs.append(T1)

            for pair in range(IMGS // 2):
                T1 = Tbufs[pair % 2]
                xf = xinpool.tile([128, NPIX], mybir.dt.float32,
                                  name="xf", tag="xf")
                nc.sync.dma_start(
                    out=xf,
                    in_=x_d[2 * pair:2 * pair + 2].rearrange(
                        "i c h w -> (i c) (h w)"))

                # u = x*sa + (MAGIC - zp)  on ACT (frees DVE for copies)
                u = wpool.tile([128, NPIX], mybir.dt.float32,
                               name="u", tag="u")
                nc.scalar.activation(out=u, in_=xf, func=F.Identity,
                                     bias=ub_sb, scale=sa)

                # A-slot interiors (bf16), one DVE op per image
                for i in range(2):
                    nc.vector.tensor_scalar(
                        T1[0:64, i:i + 1, 1:57, 1:57],
                        u[64 * i:64 * i + 64].rearrange(
                            "p (h w) -> p h w", h=H),
                        a_bias, None, op0=A.add)
                # B = A shift (1,0): covers rows 0..56 (incl. pad cols)
                nc.vector.tensor_copy(
                    T1[64:128, :, 0:57, :],
                    T1[0:64, :, 1:58, :])

                ystage = wpool.tile([128, NPIX], mybir.dt.float32,
                                    name="ystage", tag="ystage")
                for ch in range(NCHUNKS):
                    R = CHUNK_ROWS * ch
                    ps = ppool.tile([128, CHUNK], mybir.dt.float32,
                                    name="ps", tag="ps")
                    # per chunk: img0's full 6-group accumulation, then
                    # img1's (one open group per bank at a time)
                    for half, i in ((0, 0), (64, 1)):
                        for g in range(NGROUPS):
                            r0 = R if g < 3 else R + 1
                            c0 = g if g < 3 else g - 3
                            mov = T1[0:128, i:i + 1, r0:r0 + CHUNK_ROWS,
                                     c0:c0 + 56]
                            lhsT = w_sb[0:128, g * 64:(g + 1) * 64]
                            nc.tensor.matmul(ps[half:half + 64], lhsT, mov,
                                             start=(g == 0),
                                             stop=(g == NGROUPS - 1))
                    # epilogue: y = psum * recip + bias (per-partition)
                    nc.scalar.activation(
                        out=ystage[:, ch * CHUNK:(ch + 1) * CHUNK],
                        in_=ps,
                        func=F.Identity,
                        bias=b_sb, scale=recip)

                nc.sync.dma_start(
                    out=y_d[2 * pair:2 * pair + 2].rearrange(
                        "i c h w -> (i c) (h w)"),
                    in_=ystage)

    nc.compile()
    _nc_cache[key] = nc
    return nc


def _prep(x, weight, bias, scale_a, scale_w, zero_point):
    x = np.ascontiguousarray(np.asarray(x, dtype=np.float32))
    weight = np.asarray(weight, dtype=np.float32)
    bias = np.asarray(bias, dtype=np.float32)
    sa = float(np.asarray(scale_a).reshape(-1)[0])
    sw = float(np.asarray(scale_w).reshape(-1)[0])
    zp = float(np.asarray(zero_point).reshape(-1)[0])

    # activation-clip guard: reference clips round(x*sa - zp) to [0, 255].
    # For in-range data the clip is a no-op; if any value could clip,
    # pre-clamp x on the host (preserves the reference's semantics).
    amax = float(np.abs(x).max())
    if not (amax * abs(sa) < abs(zp if zp != 0 else 0) + 126.99 and
            -0.49 < -zp and sa * amax - zp < 255.49):
        f32 = np.float32
        lo = (f32(-0.49) + f32(zp)) / f32(sa)
        hi = (f32(255.49) + f32(zp)) / f32(sa)
        x = np.clip(x, lo, hi).astype(np.float32)

    # weight quantization, matching jnp.round(weight * sw) in f32 + the
    # implicit 8-bit two's-complement wrap of the bit decomposition
    qw = np.round(weight * np.float32(sw))
    qwi = qw.astype(np.int64)
    qw_eff = ((qwi + 128) % 256) - 128
    delta = qwi - qw_eff          # nonzero only if |qw| > 127 (never for
    # randn*20 weights); handled via a host-side correction plane below.

    wt = qw_eff.astype(np.float32)      # [o, i, 3, 3]

    def tap(kh, kw):
        return np.ascontiguousarray(wt[:, :, kh, kw].T)   # [in, out]

    wg = np.zeros((128, NGROUPS * 64), np.float32)
    for g in range(3):                   # {(0,g),(1,g)} pairs
        wg[0:64, g * 64:(g + 1) * 64] = tap(0, g)
        wg[64:128, g * 64:(g + 1) * 64] = tap(1, g)
    for g in range(3):                   # singles {(2,g)} via B-half
        wg[64:128, (3 + g) * 64:(4 + g) * 64] = tap(2, g)
    wg_bf = np.ascontiguousarray(wg.astype(ml_dtypes.bfloat16))

    bias_dup = np.ascontiguousarray(
        np.concatenate([bias, bias])[:, None].astype(np.float32))

    sprod = np.float32(sw) * np.float32(sa)
    recip = float(np.float32(1.0) / sprod)

    corr = None
    if np.any(delta != 0):
        # reference's zero-point term uses the unwrapped qw:
        # out_ref - out_dev = zp * conv2d(ones, delta) * recip
        dsum = delta.sum(axis=1).astype(np.float64)  # [o, 3, 3]
        plane = np.zeros((C, H, W), np.float64)
        for kh in range(3):
            for kw in range(3):
                h0, h1 = max(0, 1 - kh), min(H, H + 1 - kh)
                w0, w1 = max(0, 1 - kw), min(W, W + 1 - kw)
                plane[:, h0:h1, w0:w1] += dsum[:, kh, kw][:, None, None]
        corr = (zp * plane * float(recip)).astype(np.float32)

    return x, wg_bf, bias_dup, sa, zp, recip, corr


def _run(x, weight, bias, scale_a, scale_w, zero_point, trace=False):
    x, wg_bf, bias_dup, sa, zp, recip, corr = _prep(
        x, weight, bias, scale_a, scale_w, zero_point)
    nc = _build(sa, -zp, recip)
    n = x.shape[0]
    assert n == N_CORES * IMGS, f"expected batch {N_CORES * IMGS}, got {n}"
    in_maps = [
        {"x": np.ascontiguousarray(x[k * IMGS:(k + 1) * IMGS]),
         "wt": wg_bf, "biasd": bias_dup}
        for k in range(N_CORES)
    ]
    try:
        res = bass_utils.run_bass_kernel_spmd(
            nc, in_maps, core_ids=list(range(N_CORES)), trace=trace)
    except ModuleNotFoundError:
        # axon NTFF profile hook unavailable in this environment
        res = bass_utils.run_bass_kernel_spmd(
            nc, in_maps, core_ids=list(range(N_CORES)), trace=False)
    y = np.concatenate([res.results[k]["y"] for k in range(N_CORES)], axis=0)
    if corr is not None:
        y = y + corr[None]
    return np.ascontiguousarray(y.astype(np.float32)), res


def kernel(x, weight, bias, scale_a, scale_w, zero_point):
    y, _ = _run(x, weight, bias, scale_a, scale_w, zero_point, trace=False)
    return y


# revision 8
# speedup vs baseline: 2.2581x; 1.1449x over previous
"""Trainium2 Bass kernel for nn_ConvQuantizationWrapper.

The reference bit-slices an 8-bit quantized 3x3 conv into 32 (2-bit act x
1-bit weight) conv passes and recombines them with powers of two. That
decomposition exactly reconstructs

    out = conv2d(A, Wq) / (sa*sw) + bias
    A   = clip(round(x*sa - zp), 0, 255) + zp        (integers in [-128,127])
    Wq  = wrap_int8(round(w*sw))                     (integers in [-128,127])

in exact integer arithmetic (all partial sums < 2^24, so bf16-input
matmuls with fp32 PSUM accumulation are exact). The kernel runs one
quantized conv, data-parallel over batch (8 images per NeuronCore).

Per image pair (2 images = 128 partitions of staging):
  - ACT: u = x*sa + (MAGIC - zp)  (fused affine; MAGIC forces RNE-to-int)
  - DVE: A-slot writes  A = u - (MAGIC - zp) -> bf16 padded [58,58] frames
         in T1[0:64], plus one row-shifted copy B = A shift (1,0) into
         T1[64:128] (K=128 tap pairing needs taps (kh),(kh+1) at the same
         within-partition offset).
  - PE: 3x3 conv as 6 matmul groups per 8-row output chunk, all K=128
        (64ch x 2 kernel rows) x M=64 x N=448:
           g0..g2: taps {(0,q),(1,q)}  from T1 at (R,   q)
           g3..g5: tap  {(2,q)} via B-half at (R+1, q), A-half weights = 0
        Per chunk/bank: img0's full 6-matmul accumulation group completes,
        then img1's (never two groups open in one bank; uniform rows).
  - ACT epilogue: y = psum * (1/(sa*sw)) + bias
"""

import numpy as np
import ml_dtypes

import concourse.bacc as bacc
import concourse.mybir as mybir
import concourse.tile as tile
from concourse import bass_utils

N_CORES = 8
IMGS = 8          # images per core (batch 64 / 8 cores)
C = 64
H = W = 56
HP = 58           # padded spatial
NPIX = H * W      # 3136
CHUNK_ROWS = 8
CHUNK = CHUNK_ROWS * W   # 448 output pixels per PSUM bank
NCHUNKS = H // CHUNK_ROWS
MAGIC = 12582912.0       # 1.5 * 2**23: float32 round-to-nearest-integer trick
NGROUPS = 6

_nc_cache = {}


def _build(sa: float, neg_zp: float, recip: float):
    """Build + compile the per-core Bass kernel (cached per scalar config)."""
    key = (sa, neg_zp, recip)
    if key in _nc_cache:
        return _nc_cache[key]

    A = mybir.AluOpType
    F = mybir.ActivationFunctionType
    nc = bacc.Bacc("TRN2", target_bir_lowering=False, debug=False)
    x_d = nc.dram_tensor("x", [IMGS, C, H, W], mybir.dt.float32,
                         kind="ExternalInput").ap()
    w_d = nc.dram_tensor("wt", [128, NGROUPS * 64], mybir.dt.bfloat16,
                         kind="ExternalInput").ap()
    b_d = nc.dram_tensor("biasd", [128, 1], mybir.dt.float32,
                         kind="ExternalInput").ap()
    y_d = nc.dram_tensor("y", [IMGS, C, H, W], mybir.dt.float32,
                         kind="ExternalOutput").ap()

    # u = x*sa + (MAGIC - zp); A-slot = u - (MAGIC - zp)
    u_bias = MAGIC + neg_zp          # MAGIC - zp
    a_bias = -(MAGIC + neg_zp)

    with tile.TileContext(nc) as tc:
        with (
            tc.tile_pool(name="const", bufs=1) as cpool,
            tc.tile_pool(name="xbuf", bufs=1) as xpool,
            tc.tile_pool(name="xin", bufs=3) as xinpool,
            tc.tile_pool(name="work", bufs=2) as wpool,
            tc.tile_pool(name="psum", bufs=8, space="PSUM") as ppool,
        ):
            w_sb = cpool.tile([128, NGROUPS * 64], mybir.dt.bfloat16,
                              name="w_sb")
            nc.sync.dma_start(out=w_sb, in_=w_d)
            b_sb = cpool.tile([128, 1], mybir.dt.float32, name="b_sb")
            nc.sync.dma_start(out=b_sb, in_=b_d)
            ub_sb = cpool.tile([128, 1], mybir.dt.float32, name="ub_sb")
            nc.vector.memset(ub_sb, u_bias)
            # dummy activation: forces the ACT table load off the
            # critical path (it otherwise fires right before the first
            # real activation, after the first input DMA lands)
            warm = cpool.tile([128, 1], mybir.dt.float32, name="warm")
            nc.scalar.activation(out=warm, in_=ub_sb, func=F.Identity,
                                 bias=ub_sb, scale=1.0)

            # Persistent double-buffered layout tiles [slot-half, img, 58,58].
            # Only the A-half borders need zeroing (reads of the pad frame);
            # the B-half is fully covered by the shifted copy each pair.
            Tbufs = []
            for j in range(2):
                T1 = xpool.tile([128, 2, HP, HP], mybir.dt.bfloat16,
                                name=f"T1_{j}", tag=f"T1_{j}")
                nc.vector.memset(T1[0:64, :, 0:1, :], 0.0)    # top pad row
                nc.vector.memset(T1[0:64, :, 57:58, :], 0.0)  # bottom pad row
                nc.vector.memset(T1[0:64, :, :, 0:1], 0.0)    # left pad col
                nc.vector.memset(T1[0:64, :, :, 57:58], 0.0)  # right pad col
                Tbufs.append(T1)

            def quant_rows(T1, u, h0, h1):
                """A-slot interiors for img rows [h0,h1) + the B rows they
                enable (B row h = A row h+1 = img row h)."""
                for i in range(2):
                    nc.vector.tensor_scalar(
                        T1[0:64, i:i + 1, 1 + h0:1 + h1, 1:57],
                        u[64 * i:64 * i + 64, h0 * W:h1 * W].rearrange(
                            "p (h w) -> p h w", h=h1 - h0),
                        a_bias, None, op0=A.add)
                # B rows h0..h1-1 <- A rows h0+1..h1 (all cols, incl pad);
                # at h1 == H also copy B row 56 <- A pad row 57
                b1 = h1 if h1 < H else H + 1
                nc.vector.tensor_copy(
                    T1[64:128, :, h0:b1, :],
                    T1[0:64, :, h0 + 1:b1 + 1, :])

            def conv_chunks(T1, ystage, ch_lo, ch_hi):
                for ch in range(ch_lo, ch_hi):
                    R = CHUNK_ROWS * ch
                    ps = ppool.tile([128, CHUNK], mybir.dt.float32,
                                    name="ps", tag="ps")
                    # per chunk: img0's full 6-group accumulation, then
                    # img1's (one open group per bank at a time)
                    for half, i in ((0, 0), (64, 1)):
                        for g in range(NGROUPS):
                            r0 = R if g < 3 else R + 1
                            c0 = g if g < 3 else g - 3
                            mov = T1[0:128, i:i + 1, r0:r0 + CHUNK_ROWS,
                                     c0:c0 + 56]
                            lhsT = w_sb[0:128, g * 64:(g + 1) * 64]
                            nc.tensor.matmul(ps[half:half + 64], lhsT, mov,
                                             start=(g == 0),
                                             stop=(g == NGROUPS - 1))
                    # epilogue: y = psum * recip + bias (per-partition)
                    nc.scalar.activation(
                        out=ystage[:, ch * CHUNK:(ch + 1) * CHUNK],
                        in_=ps,
                        func=F.Identity,
                        bias=b_sb, scale=recip)

            for pair in range(IMGS // 2):
                T1 = Tbufs[pair % 2]
                xf = xinpool.tile([128, NPIX], mybir.dt.float32,
                                  name="xf", tag="xf")
                x_ap = x_d[2 * pair:2 * pair + 2].rearrange(
                    "i c h w -> (i c) (h w)")
                y_ap = y_d[2 * pair:2 * pair + 2].rearrange(
                    "i c h w -> (i c) (h w)")
                u = wpool.tile([128, NPIX], mybir.dt.float32,
                               name="u", tag="u")
                ystage = wpool.tile([128, NPIX], mybir.dt.float32,
                                    name="ystage", tag="ystage")
                HSPL = 32            # row split point (chunk boundary)
                CSPL = HSPL // CHUNK_ROWS
                if pair == 0:
                    # split the first pair's load + quant by row halves so
                    # matmuls start before the full image is staged
                    nc.sync.dma_start(out=xf[:, :HSPL * W],
                                      in_=x_ap[:, :HSPL * W])
                    nc.sync.dma_start(out=xf[:, HSPL * W:],
                                      in_=x_ap[:, HSPL * W:])
                    nc.scalar.activation(out=u[:, :HSPL * W],
                                         in_=xf[:, :HSPL * W],
                                         func=F.Identity,
                                         bias=ub_sb, scale=sa)
                    quant_rows(T1, u, 0, HSPL)
                    # chunk CSPL-1 needs B row HSPL (second quant half),
                    # so convolve only chunks 0..CSPL-2 here
                    conv_chunks(T1, ystage, 0, CSPL - 1)
                    nc.scalar.activation(out=u[:, HSPL * W:],
                                         in_=xf[:, HSPL * W:],
                                         func=F.Identity,
                                         bias=ub_sb, scale=sa)
                    quant_rows(T1, u, HSPL, H)
                    conv_chunks(T1, ystage, CSPL - 1, NCHUNKS)
                else:
                    nc.sync.dma_start(out=xf, in_=x_ap)
                    # u = x*sa + (MAGIC - zp) on ACT (frees DVE for copies)
                    nc.scalar.activation(out=u, in_=xf, func=F.Identity,
                                         bias=ub_sb, scale=sa)
                    quant_rows(T1, u, 0, H)
                    conv_chunks(T1, ystage, 0, CSPL)
                    nc.sync.dma_start(out=y_ap[:, :CSPL * CHUNK],
                                      in_=ystage[:, :CSPL * CHUNK])
                    conv_chunks(T1, ystage, CSPL, NCHUNKS)
                nc.sync.dma_start(out=y_ap[:, CSPL * CHUNK:],
                                  in_=ystage[:, CSPL * CHUNK:])
                if pair == 0:
                    nc.sync.dma_start(out=y_ap[:, :CSPL * CHUNK],
                                      in_=ystage[:, :CSPL * CHUNK])

    nc.compile()
    _nc_cache[key] = nc
    return nc


def _prep(x, weight, bias, scale_a, scale_w, zero_point):
    x = np.ascontiguousarray(np.asarray(x, dtype=np.float32))
    weight = np.asarray(weight, dtype=np.float32)
    bias = np.asarray(bias, dtype=np.float32)
    sa = float(np.asarray(scale_a).reshape(-1)[0])
    sw = float(np.asarray(scale_w).reshape(-1)[0])
    zp = float(np.asarray(zero_point).reshape(-1)[0])

    # activation-clip guard: reference clips round(x*sa - zp) to [0, 255].
    # For in-range data the clip is a no-op; if any value could clip,
    # pre-clamp x on the host (preserves the reference's semantics).
    amax = float(np.abs(x).max())
    if not (amax * abs(sa) < abs(zp if zp != 0 else 0) + 126.99 and
            -0.49 < -zp and sa * amax - zp < 255.49):
        f32 = np.float32
        lo = (f32(-0.49) + f32(zp)) / f32(sa)
        hi = (f32(255.49) + f32(zp)) / f32(sa)
        x = np.clip(x, lo, hi).astype(np.float32)

    # weight quantization, matching jnp.round(weight * sw) in f32 + the
    # implicit 8-bit two's-complement wrap of the bit decomposition
    qw = np.round(weight * np.float32(sw))
    qwi = qw.astype(np.int64)
    qw_eff = ((qwi + 128) % 256) - 128
    delta = qwi - qw_eff          # nonzero only if |qw| > 127 (never for
    # randn*20 weights); handled via a host-side correction plane below.

    wt = qw_eff.astype(np.float32)      # [o, i, 3, 3]

    def tap(kh, kw):
        return np.ascontiguousarray(wt[:, :, kh, kw].T)   # [in, out]

    wg = np.zeros((128, NGROUPS * 64), np.float32)
    for g in range(3):                   # {(0,g),(1,g)} pairs
        wg[0:64, g * 64:(g + 1) * 64] = tap(0, g)
        wg[64:128, g * 64:(g + 1) * 64] = tap(1, g)
    for g in range(3):                   # singles {(2,g)} via B-half
        wg[64:128, (3 + g) * 64:(4 + g) * 64] = tap(2, g)
    wg_bf = np.ascontiguousarray(wg.astype(ml_dtypes.bfloat16))

    bias_dup = np.ascontiguousarray(
        np.concatenate([bias, bias])[:, None].astype(np.float32))

    sprod = np.float32(sw) * np.float32(sa)
    recip = float(np.float32(1.0) / sprod)

    corr = None
    if np.any(delta != 0):
        # reference's zero-point term uses the unwrapped qw:
        # out_ref - out_dev = zp * conv2d(ones, delta) * recip
        dsum = delta.sum(axis=1).astype(np.float64)  # [o, 3, 3]
        plane = np.zeros((C, H, W), np.float64)
        for kh in range(3):
            for kw in range(3):
                h0, h1 = max(0, 1 - kh), min(H, H + 1 - kh)
                w0, w1 = max(0, 1 - kw), min(W, W + 1 - kw)
                plane[:, h0:h1, w0:w1] += dsum[:, kh, kw][:, None, None]
        corr = (zp * plane * float(recip)).astype(np.float32)

    return x, wg_bf, bias_dup, sa, zp, recip, corr


def _run(x, weight, bias, scale_a, scale_w, zero_point, trace=False):
    x, wg_bf, bias_dup, sa, zp, recip, corr = _prep(
        x, weight, bias, scale_a, scale_w, zero_point)
    nc = _build(sa, -zp, recip)
    n = x.shape[0]
    assert n == N_CORES * IMGS, f"expected batch {N_CORES * IMGS}, got {n}"
    in_maps = [
        {"x": np.ascontiguousarray(x[k * IMGS:(k + 1) * IMGS]),
         "wt": wg_bf, "biasd": bias_dup}
        for k in range(N_CORES)
    ]
    try:
        res = bass_utils.run_bass_kernel_spmd(
            nc, in_maps, core_ids=list(range(N_CORES)), trace=trace)
    except ModuleNotFoundError:
        # axon NTFF profile hook unavailable in this environment
        res = bass_utils.run_bass_kernel_spmd(
            nc, in_maps, core_ids=list(range(N_CORES)), trace=False)
    y = np.concatenate([res.results[k]["y"] for k in range(N_CORES)], axis=0)
    if corr is not None:
        y = y + corr[None]
    return np.ascontiguousarray(y.astype(np.float32)), res


def kernel(x, weight, bias, scale_a, scale_w, zero_point):
    y, _ = _run(x, weight, bias, scale_a, scale_w, zero_point, trace=False)
    return y
